# revision 14
# baseline (speedup 1.0000x reference)
"""Trainium2 Bass kernel for nn_AttentionResidual (sparse_attention).

Computes, for V:(n=8,b=4,s=2048,d=1024), proj:(12,1024), scale:(1024,), block_idx:
    w       = proj[min(block_idx, 11)]
    rms     = sqrt(mean(V^2, axis=-1) + 1e-5)
    logits  = sum_d (w*scale)[d] * V[...,d] / rms
    weights = softmax(logits, axis=n)
    out     = sum_n weights[n] * V[n]                       # (b,s,d)

Sharding: data-parallel over the 8192 (b,s) positions across 8 NeuronCores
(1024 positions per core). proj/scale fold into one d-vector on the host.

v2 design (fp16): V is shipped fp16 in [block, pos, n, d] layout so each
128-position block is ONE 2 MiB DMA (16 KiB/partition contiguous). This
halves HBM traffic (the f32 roofline was ~107us; fp16 is ~53us) at ~7e-3
relative error, well inside the 2e-2 gate. Per block:
  - sum-of-squares and ws-dot reductions are split across ACT (Square+accum),
    DVE (tensor_scalar pow / STT + accum), and GPSIMD/Pool (STT+accum)
    per the SOS_ENG/DOT_ENG tables (DVE STT is 1x regardless of dtype;
    ACT has no 16-bit speedup; Pool is ~2x slower than DVE 1x - so the
    three-way split is what hides compute under the DMA floor).
  - softmax stats on [128,8] tiles (ACT Ln/Exp chain + DVE smalls)
  - weighted sum on the TensorEngine: diag(e_n) built by DVE tensor_scalar
    (4x fp16), 8 accumulating fp16 matmuls per PSUM half-bank pair; ACT
    copies PSUM->SBUF fp16 with the 1/sum(e) normalization folded in.
  - output DMA'd fp16 (host upcasts).
"""

import numpy as np

N, B, S, D = 8, 4, 2048, 1024
NCORES = 8
BS = B * S            # 8192 flattened (b,s) positions
PER = BS // NCORES    # 1024 positions per core
PB = PER // 128       # 8 position blocks per core
ND = N * D            # 8192 (n,d) elements per position
EPS = 1e-5

# Per-(block parity, n) engine assignment. A=ACT, V=DVE, P=Pool.
# Measured unit costs (fp16 [128,1024]): ACT Square+accum ~1266ns;
# DVE STT+accum ~1224ns (STT has no 2x modes); Pool TT-mult (~2.2us) +
# Pool tensor_reduce (~1us) - slow but the engine is otherwise idle.
SOS_ENG = ["AAAAAAAA"] * 6 + ["VAAAAAAA"] * 2  # per block
DOT_ENG = ["VVVVVVVV"] * 8

_cache = {}


def _build():
    import concourse.tile as tile
    from concourse import bacc, mybir

    OP = mybir.AluOpType
    A = mybir.ActivationFunctionType
    X = mybir.AxisListType.X
    f32 = mybir.dt.float32
    f16 = mybir.dt.float16

    from concourse.hw_specs import get_activation_tables

    nc = bacc.Bacc(
        "TRN2",
        target_bir_lowering=False,
        debug=False,
        enable_asserts=False,
        num_devices=NCORES,
    )
    v = nc.dram_tensor("v", [PB, 128, ND], f16, kind="ExternalInput").ap()
    wsb = nc.dram_tensor("wsb", [128, D], f16, kind="ExternalInput").ap()
    ident = nc.dram_tensor("ident", [128, 128], f16, kind="ExternalInput").ap()
    didx = nc.dram_tensor("didx", [128, N], mybir.dt.int16, kind="ExternalInput").ap()
    o = nc.dram_tensor("o", [PER, D], f16, kind="ExternalOutput").ap()

    # One ACT table set covers Square/Ln/Exp/Copy; pre-place its load so the
    # bacc pass doesn't ping-pong between smaller sets.
    act_set_id = list(get_activation_tables(nc.m.arch).keys()).index(
        "natural_log_exp_and_others"
    )

    with tile.TileContext(nc) as tc:
        with (
            tc.tile_pool(name="vp", bufs=4) as vp,
            tc.tile_pool(name="wp", bufs=1) as wp,
            tc.tile_pool(name="scrA", bufs=2) as scrA,
            tc.tile_pool(name="scrV", bufs=2) as scrV,
            tc.tile_pool(name="scrP", bufs=2) as scrP,
            tc.tile_pool(name="st", bufs=8) as st,
            tc.tile_pool(name="dg", bufs=3) as dgp,
            tc.tile_pool(name="ac", bufs=3) as ac,
            tc.tile_pool(name="ps", bufs=3, space="PSUM") as ps,
        ):
            nc.scalar.add_instruction(
                mybir.InstLoadActFuncSet(
                    name=nc.get_next_instruction_name(),
                    ins=[],
                    outs=[],
                    act_func_set_id=act_set_id,
                )
            )
            wt = wp.tile([128, D], f16, tag="w")
            nc.sync.dma_start(wt[:], wsb[:])
            idt = wp.tile([128, 128], f16, tag="id")
            nc.sync.dma_start(idt[:], ident[:])
            didxt = wp.tile([128, N], mybir.dt.int16, tag="didx")
            nc.sync.dma_start(didxt[:], didx[:])
            epsb = wp.tile([128, 1], f32, tag="eps")
            nc.vector.memset(epsb[:], EPS)

            # Skewed software pipeline, one iteration per 128-position
            # block. In-order engine queues mean a dependency ping-pong
            # (ss -> Ln -> y0 -> lg -> nm -> e -> scatter -> matmul -> copy)
            # stalls every engine if issued densely per block; instead each
            # stage is issued one block behind the stage it depends on, so
            # every queued op's inputs are already complete when reached:
            #   iter pp: DVE[lg,nm(pp-1)] | reductions(pp) | DVE[rs(pp-1)]
            #            ACT[Ln,y0(pp)] ACT[e(pp-1)] Pool[scatter(pp-1)]
            #            PE[matmuls(pp-1)] ACT[copy(pp-2)]
            blk = {}
            for pp in range(PB + 2):
                if pp >= 1 and pp - 1 < PB:
                    b = blk[pp - 1]
                    nc.vector.tensor_mul(b["lg"][:], b["dot"][:], b["y0"][:])
                    nc.vector.tensor_reduce(
                        b["nm"][:], b["lg"][:], X, OP.max, negate=True
                    )
                if pp < PB:
                    sos_eng = SOS_ENG[pp]
                    t = vp.tile([128, ND], f16, tag="v", name=f"v_{pp}")
                    nc.sync.dma_start(t[:], v[pp, :, :])
                    ss = st.tile([128, N], f32, tag="ss", name=f"ss_{pp}")
                    dot = st.tile([128, N], f32, tag="dot", name=f"dot_{pp}")
                    for n in range(N):
                        vn = t[:, n * D : (n + 1) * D]
                        if sos_eng[n] == "A":
                            sq = scrA.tile([128, D], f16, tag="sqA")
                            nc.scalar.activation(
                                sq[:], vn, A.Square,
                                accum_out=ss[:, n : n + 1],
                            )
                        else:
                            sq = scrV.tile([128, D], f16, tag="sqV")
                            nc.vector.scalar_tensor_tensor(
                                out=sq[:], in0=vn, scalar=1.0, in1=vn,
                                op0=OP.mult, op1=OP.mult,
                                accum_out=ss[:, n : n + 1],
                            )
                        td = scrV.tile([128, D], f16, tag="tdV")
                        nc.vector.scalar_tensor_tensor(
                            out=td[:], in0=vn, scalar=1.0, in1=wt[:],
                            op0=OP.mult, op1=OP.mult,
                            accum_out=dot[:, n : n + 1],
                        )
                    lnt = st.tile([128, N], f32, tag="lnt", name=f"ln_{pp}")
                    nc.scalar.activation(
                        lnt[:], ss[:], A.Ln, bias=epsb[:], scale=1.0 / D
                    )
                    y0 = st.tile([128, N], f32, tag="y0", name=f"y0_{pp}")
                    nc.scalar.activation(y0[:], lnt[:], A.Exp, scale=-0.5)
                    blk[pp] = {
                        "t": t, "dot": dot, "y0": y0,
                        "lg": st.tile([128, N], f32, tag="lg", name=f"lg_{pp}"),
                        "nm": st.tile([128, 1], f32, tag="nm", name=f"nm_{pp}"),
                    }
                if pp >= 1 and pp - 1 < PB:
                    b = blk[pp - 1]
                    e = st.tile([128, N], f16, tag="e", name=f"e_{pp - 1}")
                    sume = st.tile([128, 1], f32, tag="sume", name=f"su_{pp - 1}")
                    nc.scalar.activation(
                        e[:], b["lg"][:], A.Exp, bias=b["nm"][:],
                        accum_out=sume[:],
                    )
                    rs = st.tile([128, 1], f32, tag="rs", name=f"rs_{pp - 1}")
                    nc.vector.reciprocal(rs[:], sume[:])
                    b["rs"] = rs
                    dgall = dgp.tile(
                        [128, N * 128], f16, tag="dg", name=f"dg_{pp - 1}"
                    )
                    nc.gpsimd.local_scatter(
                        dgall[:], e[:], didxt[:],
                        channels=128, num_elems=N * 128, num_idxs=N,
                    )
                    acc_ps = ps.tile([128, D], f32, tag="acc", name=f"ps_{pp - 1}")
                    tq = b["t"]
                    for n in range(N):
                        dg_n = dgall[:, n * 128 : (n + 1) * 128]
                        nc.tensor.matmul(
                            acc_ps[:, 0:512], dg_n,
                            tq[:, n * D : n * D + 512],
                            start=(n == 0), stop=(n == N - 1),
                        )
                        nc.tensor.matmul(
                            acc_ps[:, 512:1024], dg_n,
                            tq[:, n * D + 512 : (n + 1) * D],
                            start=(n == 0), stop=(n == N - 1),
                        )
                    b["ps"] = acc_ps
                if pp >= 2:
                    qp = pp - 2
                    b = blk.pop(qp)
                    acc = ac.tile([128, D], f16, tag="acc_sb")
                    nc.scalar.activation(
                        acc[:], b["ps"][:], A.Copy, scale=b["rs"][:]
                    )
                    nc.sync.dma_start(o[qp * 128 : (qp + 1) * 128, :], acc[:])

    nc.compile()
    return nc


def get_program():
    if "nc" not in _cache:
        _cache["nc"] = _build()
    return _cache["nc"]


def make_in_maps(V, proj, scale, block_idx):
    V = np.asarray(V, dtype=np.float32)
    proj = np.asarray(proj, dtype=np.float32)
    scale = np.asarray(scale, dtype=np.float32)
    idx = min(int(block_idx), proj.shape[0] - 1)
    ws = (proj[idx] * scale).astype(np.float16)
    wsb = np.ascontiguousarray(np.broadcast_to(ws, (128, D)))
    eye = np.eye(128, dtype=np.float16)
    didx = (
        np.arange(N, dtype=np.int16)[None, :] * 128
        + np.arange(128, dtype=np.int16)[:, None]
    ).astype(np.int16)
    # [N, BS, D] -> [NCORES, PB, 128, N, D] fp16
    Vp = (
        V.reshape(N, NCORES, PB, 128, D)
        .transpose(1, 2, 3, 0, 4)
        .astype(np.float16)
    )
    return [
        {
            "v": np.ascontiguousarray(Vp[k]).reshape(PB, 128, ND),
            "wsb": wsb,
            "ident": eye,
            "didx": didx,
        }
        for k in range(NCORES)
    ]


def kernel(V, proj, scale, block_idx):
    from concourse.bass_utils import run_bass_kernel_spmd

    nc = get_program()
    in_maps = make_in_maps(V, proj, scale, block_idx)
    res = run_bass_kernel_spmd(nc, in_maps, core_ids=list(range(NCORES)))
    _cache["last_exec_time_ns"] = res.exec_time_ns
    _cache["last_results"] = res
    out = np.concatenate(
        [res.results[k]["o"].astype(np.float32) for k in range(NCORES)], axis=0
    )
    return out.reshape(B, S, D)


# revision 15
# speedup vs baseline: 1.0062x; 1.0062x over previous
"""Trainium2 Bass kernel for nn_AttentionResidual (sparse_attention).

Computes, for V:(n=8,b=4,s=2048,d=1024), proj:(12,1024), scale:(1024,), block_idx:
    w       = proj[min(block_idx, 11)]
    rms     = sqrt(mean(V^2, axis=-1) + 1e-5)
    logits  = sum_d (w*scale)[d] * V[...,d] / rms
    weights = softmax(logits, axis=n)
    out     = sum_n weights[n] * V[n]                       # (b,s,d)

Sharding: data-parallel over the 8192 (b,s) positions across 8 NeuronCores
(1024 positions per core). proj/scale fold into one d-vector on the host.

v2 design (fp16): V is shipped fp16 in [block, pos, n, d] layout so each
128-position block is ONE 2 MiB DMA (16 KiB/partition contiguous). This
halves HBM traffic (the f32 roofline was ~107us; fp16 is ~53us) at ~7e-3
relative error, well inside the 2e-2 gate. Per block:
  - sum-of-squares and ws-dot reductions are split across ACT (Square+accum),
    DVE (tensor_scalar pow / STT + accum), and GPSIMD/Pool (STT+accum)
    per the SOS_ENG/DOT_ENG tables (DVE STT is 1x regardless of dtype;
    ACT has no 16-bit speedup; Pool is ~2x slower than DVE 1x - so the
    three-way split is what hides compute under the DMA floor).
  - softmax stats on [128,8] tiles (ACT Ln/Exp chain + DVE smalls)
  - weighted sum on the TensorEngine: diag(e_n) built by DVE tensor_scalar
    (4x fp16), 8 accumulating fp16 matmuls per PSUM half-bank pair; ACT
    copies PSUM->SBUF fp16 with the 1/sum(e) normalization folded in.
  - output DMA'd fp16 (host upcasts).
"""

import numpy as np

N, B, S, D = 8, 4, 2048, 1024
NCORES = 8
BS = B * S            # 8192 flattened (b,s) positions
PER = BS // NCORES    # 1024 positions per core
PB = PER // 128       # 8 position blocks per core
ND = N * D            # 8192 (n,d) elements per position
EPS = 1e-5

# Per-(block parity, n) engine assignment. A=ACT, V=DVE, P=Pool.
# Measured unit costs (fp16 [128,1024]): ACT Square+accum ~1266ns;
# DVE STT+accum ~1224ns (STT has no 2x modes); Pool TT-mult (~2.2us) +
# Pool tensor_reduce (~1us) - slow but the engine is otherwise idle.
SOS_ENG = ["AAAAAAAA"] * 6 + ["VAAAAAAA"] * 2  # per block
DOT_ENG = ["VVVVVVVV"] * 8

_cache = {}


def _build():
    import concourse.tile as tile
    from concourse import bacc, mybir

    OP = mybir.AluOpType
    A = mybir.ActivationFunctionType
    X = mybir.AxisListType.X
    f32 = mybir.dt.float32
    f16 = mybir.dt.float16

    from concourse.hw_specs import get_activation_tables

    nc = bacc.Bacc(
        "TRN2",
        target_bir_lowering=False,
        debug=False,
        enable_asserts=False,
        num_devices=NCORES,
    )
    v = nc.dram_tensor("v", [PB, 128, ND], f16, kind="ExternalInput").ap()
    wsb = nc.dram_tensor("wsb", [128, D], f16, kind="ExternalInput").ap()
    ident = nc.dram_tensor("ident", [128, 128], f16, kind="ExternalInput").ap()
    didx = nc.dram_tensor("didx", [128, N], mybir.dt.int16, kind="ExternalInput").ap()
    o = nc.dram_tensor("o", [PER, D], f16, kind="ExternalOutput").ap()

    # One ACT table set covers Square/Ln/Exp/Copy; pre-place its load so the
    # bacc pass doesn't ping-pong between smaller sets.
    act_set_id = list(get_activation_tables(nc.m.arch).keys()).index(
        "natural_log_exp_and_others"
    )

    with tile.TileContext(nc) as tc:
        with (
            tc.tile_pool(name="vp", bufs=4) as vp,
            tc.tile_pool(name="wp", bufs=1) as wp,
            tc.tile_pool(name="scrA", bufs=2) as scrA,
            tc.tile_pool(name="scrV", bufs=2) as scrV,
            tc.tile_pool(name="scrP", bufs=2) as scrP,
            tc.tile_pool(name="st", bufs=8) as st,
            tc.tile_pool(name="dg", bufs=3) as dgp,
            tc.tile_pool(name="ac", bufs=3) as ac,
            tc.tile_pool(name="ps", bufs=3, space="PSUM") as ps,
        ):
            nc.scalar.add_instruction(
                mybir.InstLoadActFuncSet(
                    name=nc.get_next_instruction_name(),
                    ins=[],
                    outs=[],
                    act_func_set_id=act_set_id,
                )
            )
            wt = wp.tile([128, D], f16, tag="w")
            nc.sync.dma_start(wt[:], wsb[:])
            idt = wp.tile([128, 128], f16, tag="id")
            nc.sync.dma_start(idt[:], ident[:])
            didxt = wp.tile([128, N], mybir.dt.int16, tag="didx")
            nc.sync.dma_start(didxt[:], didx[:])
            epsb = wp.tile([128, 1], f32, tag="eps")
            nc.vector.memset(epsb[:], EPS)

            # Skewed software pipeline, one iteration per 128-position
            # block. In-order engine queues mean a dependency ping-pong
            # (ss -> Ln -> y0 -> lg -> nm -> e -> scatter -> matmul -> copy)
            # stalls every engine if issued densely per block; instead each
            # stage is issued one block behind the stage it depends on, so
            # every queued op's inputs are already complete when reached:
            #   iter pp: DVE[lg,nm(pp-1)] | reductions(pp) | DVE[rs(pp-1)]
            #            ACT[Ln,y0(pp)] ACT[e(pp-1)] Pool[scatter(pp-1)]
            #            PE[matmuls(pp-1)] ACT[copy(pp-2)]
            blk = {}
            for pp in range(PB + 2):
                if pp >= 1 and pp - 1 < PB:
                    b = blk[pp - 1]
                    e = st.tile([128, N], f16, tag="e", name=f"e_{pp - 1}")
                    sume = st.tile([128, 1], f32, tag="sume", name=f"su_{pp - 1}")
                    nc.scalar.activation(
                        e[:], b["lg"][:], A.Exp, bias=b["nm"][:],
                        accum_out=sume[:],
                    )
                    b["sume"] = sume
                    dgall = dgp.tile(
                        [128, N * 128], f16, tag="dg", name=f"dg_{pp - 1}"
                    )
                    nc.gpsimd.local_scatter(
                        dgall[:], e[:], didxt[:],
                        channels=128, num_elems=N * 128, num_idxs=N,
                    )
                    acc_ps = ps.tile([128, D], f32, tag="acc", name=f"ps_{pp - 1}")
                    tq = b["t"]
                    for n in range(N):
                        dg_n = dgall[:, n * 128 : (n + 1) * 128]
                        nc.tensor.matmul(
                            acc_ps[:, 0:512], dg_n,
                            tq[:, n * D : n * D + 512],
                            start=(n == 0), stop=(n == N - 1),
                        )
                        nc.tensor.matmul(
                            acc_ps[:, 512:1024], dg_n,
                            tq[:, n * D + 512 : (n + 1) * D],
                            start=(n == 0), stop=(n == N - 1),
                        )
                    b["ps"] = acc_ps
                if pp < PB:
                    sos_eng = SOS_ENG[pp]
                    t = vp.tile([128, ND], f16, tag="v", name=f"v_{pp}")
                    if pp == 0:
                        # split the cold-start DMA so the first squares can
                        # begin ~4x earlier
                        for q in range(4):
                            nc.sync.dma_start(
                                t[:, q * (ND // 4) : (q + 1) * (ND // 4)],
                                v[pp, :, q * (ND // 4) : (q + 1) * (ND // 4)],
                            )
                    else:
                        nc.sync.dma_start(t[:], v[pp, :, :])
                    ss = st.tile([128, N], f32, tag="ss", name=f"ss_{pp}")
                    dot = st.tile([128, N], f32, tag="dot", name=f"dot_{pp}")
                    for n in range(N):
                        vn = t[:, n * D : (n + 1) * D]
                        if sos_eng[n] == "A":
                            sq = scrA.tile([128, D], f16, tag="sqA")
                            nc.scalar.activation(
                                sq[:], vn, A.Square,
                                accum_out=ss[:, n : n + 1],
                            )
                        else:
                            sq = scrV.tile([128, D], f16, tag="sqV")
                            nc.vector.scalar_tensor_tensor(
                                out=sq[:], in0=vn, scalar=1.0, in1=vn,
                                op0=OP.mult, op1=OP.mult,
                                accum_out=ss[:, n : n + 1],
                            )
                        td = scrV.tile([128, D], f16, tag="tdV")
                        nc.vector.scalar_tensor_tensor(
                            out=td[:], in0=vn, scalar=1.0, in1=wt[:],
                            op0=OP.mult, op1=OP.mult,
                            accum_out=dot[:, n : n + 1],
                        )
                    lnt = st.tile([128, N], f32, tag="lnt", name=f"ln_{pp}")
                    nc.scalar.activation(
                        lnt[:], ss[:], A.Ln, bias=epsb[:], scale=1.0 / D
                    )
                    y0 = st.tile([128, N], f32, tag="y0", name=f"y0_{pp}")
                    nc.scalar.activation(y0[:], lnt[:], A.Exp, scale=-0.5)
                    blk[pp] = {
                        "t": t, "dot": dot, "y0": y0,
                        "lg": st.tile([128, N], f32, tag="lg", name=f"lg_{pp}"),
                        "nm": st.tile([128, 1], f32, tag="nm", name=f"nm_{pp}"),
                    }
                if pp >= 1 and pp - 1 < PB:
                    b = blk[pp - 1]
                    rs = st.tile([128, 1], f32, tag="rs", name=f"rs_{pp - 1}")
                    nc.vector.reciprocal(rs[:], b["sume"][:])
                    b["rs"] = rs
                if pp < PB:
                    b = blk[pp]
                    nc.vector.tensor_mul(b["lg"][:], b["dot"][:], b["y0"][:])
                    nc.vector.tensor_reduce(
                        b["nm"][:], b["lg"][:], X, OP.max, negate=True
                    )
                if pp >= 2:
                    qp = pp - 2
                    b = blk.pop(qp)
                    acc = ac.tile([128, D], f16, tag="acc_sb")
                    nc.scalar.activation(
                        acc[:], b["ps"][:], A.Copy, scale=b["rs"][:]
                    )
                    nc.gpsimd.dma_start(o[qp * 128 : (qp + 1) * 128, :], acc[:])

    nc.compile()
    return nc


def get_program():
    if "nc" not in _cache:
        _cache["nc"] = _build()
    return _cache["nc"]


def make_in_maps(V, proj, scale, block_idx):
    V = np.asarray(V, dtype=np.float32)
    proj = np.asarray(proj, dtype=np.float32)
    scale = np.asarray(scale, dtype=np.float32)
    idx = min(int(block_idx), proj.shape[0] - 1)
    ws = (proj[idx] * scale).astype(np.float16)
    wsb = np.ascontiguousarray(np.broadcast_to(ws, (128, D)))
    eye = np.eye(128, dtype=np.float16)
    didx = (
        np.arange(N, dtype=np.int16)[None, :] * 128
        + np.arange(128, dtype=np.int16)[:, None]
    ).astype(np.int16)
    # [N, BS, D] -> [NCORES, PB, 128, N, D] fp16
    Vp = (
        V.reshape(N, NCORES, PB, 128, D)
        .transpose(1, 2, 3, 0, 4)
        .astype(np.float16)
    )
    return [
        {
            "v": np.ascontiguousarray(Vp[k]).reshape(PB, 128, ND),
            "wsb": wsb,
            "ident": eye,
            "didx": didx,
        }
        for k in range(NCORES)
    ]


def kernel(V, proj, scale, block_idx):
    from concourse.bass_utils import run_bass_kernel_spmd

    nc = get_program()
    in_maps = make_in_maps(V, proj, scale, block_idx)
    res = run_bass_kernel_spmd(nc, in_maps, core_ids=list(range(NCORES)))
    _cache["last_exec_time_ns"] = res.exec_time_ns
    _cache["last_results"] = res
    out = np.concatenate(
        [res.results[k]["o"].astype(np.float32) for k in range(NCORES)], axis=0
    )
    return out.reshape(B, S, D)


# revision 16
# speedup vs baseline: 1.0254x; 1.0190x over previous
"""Trainium2 Bass kernel for nn_AttentionResidual (sparse_attention).

Computes, for V:(n=8,b=4,s=2048,d=1024), proj:(12,1024), scale:(1024,), block_idx:
    w       = proj[min(block_idx, 11)]
    rms     = sqrt(mean(V^2, axis=-1) + 1e-5)
    logits  = sum_d (w*scale)[d] * V[...,d] / rms
    weights = softmax(logits, axis=n)
    out     = sum_n weights[n] * V[n]                       # (b,s,d)

Sharding: data-parallel over the 8192 (b,s) positions across 8 NeuronCores
(1024 positions per core). proj/scale fold into one d-vector on the host.

v2 design (fp16): V is shipped fp16 in [block, pos, n, d] layout so each
128-position block is ONE 2 MiB DMA (16 KiB/partition contiguous). This
halves HBM traffic (the f32 roofline was ~107us; fp16 is ~53us) at ~7e-3
relative error, well inside the 2e-2 gate. Per block:
  - sum-of-squares and ws-dot reductions are split across ACT (Square+accum),
    DVE (tensor_scalar pow / STT + accum), and GPSIMD/Pool (STT+accum)
    per the SOS_ENG/DOT_ENG tables (DVE STT is 1x regardless of dtype;
    ACT has no 16-bit speedup; Pool is ~2x slower than DVE 1x - so the
    three-way split is what hides compute under the DMA floor).
  - softmax stats on [128,8] tiles (ACT Ln/Exp chain + DVE smalls)
  - weighted sum on the TensorEngine: diag(e_n) built by DVE tensor_scalar
    (4x fp16), 8 accumulating fp16 matmuls per PSUM half-bank pair; ACT
    copies PSUM->SBUF fp16 with the 1/sum(e) normalization folded in.
  - output DMA'd fp16 (host upcasts).
"""

import numpy as np

N, B, S, D = 8, 4, 2048, 1024
NCORES = 8
BS = B * S            # 8192 flattened (b,s) positions
PER = BS // NCORES    # 1024 positions per core
PB = PER // 128       # 8 position blocks per core
ND = N * D            # 8192 (n,d) elements per position
EPS = 1e-5

# Per-(block parity, n) engine assignment. A=ACT, V=DVE, P=Pool.
# Measured unit costs (fp16 [128,1024]): ACT Square+accum ~1266ns;
# DVE STT+accum ~1224ns (STT has no 2x modes); Pool TT-mult (~2.2us) +
# Pool tensor_reduce (~1us) - slow but the engine is otherwise idle.
SOS_ENG = ["AAAAAAAA"] * 6 + ["VAAAAAAA"] * 2  # per block
DOT_ENG = ["VVVVVVVV"] * 8

_cache = {}


def _build():
    import concourse.tile as tile
    from concourse import bacc, mybir

    OP = mybir.AluOpType
    A = mybir.ActivationFunctionType
    X = mybir.AxisListType.X
    f32 = mybir.dt.float32
    f16 = mybir.dt.float16

    from concourse.hw_specs import get_activation_tables

    nc = bacc.Bacc(
        "TRN2",
        target_bir_lowering=False,
        debug=False,
        enable_asserts=False,
        num_devices=NCORES,
    )
    v = nc.dram_tensor("v", [PB, 128, ND], f16, kind="ExternalInput").ap()
    wsb = nc.dram_tensor("wsb", [128, D], f16, kind="ExternalInput").ap()
    ident = nc.dram_tensor("ident", [128, 128], f16, kind="ExternalInput").ap()
    didx = nc.dram_tensor("didx", [128, N], mybir.dt.int16, kind="ExternalInput").ap()
    o = nc.dram_tensor("o", [PER, D], f16, kind="ExternalOutput").ap()

    # One ACT table set covers Square/Ln/Exp/Copy; pre-place its load so the
    # bacc pass doesn't ping-pong between smaller sets.
    act_set_id = list(get_activation_tables(nc.m.arch).keys()).index(
        "natural_log_exp_and_others"
    )

    with tile.TileContext(nc) as tc:
        with (
            tc.tile_pool(name="vp", bufs=4) as vp,
            tc.tile_pool(name="wp", bufs=1) as wp,
            tc.tile_pool(name="scrA", bufs=2) as scrA,
            tc.tile_pool(name="scrV", bufs=2) as scrV,
            tc.tile_pool(name="scrP", bufs=2) as scrP,
            tc.tile_pool(name="st", bufs=8) as st,
            tc.tile_pool(name="dg", bufs=3) as dgp,
            tc.tile_pool(name="ac", bufs=3) as ac,
            tc.tile_pool(name="ps", bufs=3, space="PSUM") as ps,
        ):
            nc.scalar.add_instruction(
                mybir.InstLoadActFuncSet(
                    name=nc.get_next_instruction_name(),
                    ins=[],
                    outs=[],
                    act_func_set_id=act_set_id,
                )
            )
            wt = wp.tile([128, D], f16, tag="w")
            idt = wp.tile([128, 128], f16, tag="id")
            didxt = wp.tile([128, N], mybir.dt.int16, tag="didx")
            epsb = wp.tile([128, 1], f32, tag="eps")
            nc.vector.memset(epsb[:], EPS)

            # Skewed software pipeline, one iteration per 128-position
            # block. In-order engine queues mean a dependency ping-pong
            # (ss -> Ln -> y0 -> lg -> nm -> e -> scatter -> matmul -> copy)
            # stalls every engine if issued densely per block; instead each
            # stage is issued one block behind the stage it depends on, so
            # every queued op's inputs are already complete when reached:
            #   iter pp: DVE[lg,nm(pp-1)] | reductions(pp) | DVE[rs(pp-1)]
            #            ACT[Ln,y0(pp)] ACT[e(pp-1)] Pool[scatter(pp-1)]
            #            PE[matmuls(pp-1)] ACT[copy(pp-2)]
            blk = {}
            for pp in range(PB + 2):
                if pp >= 1 and pp - 1 < PB:
                    b = blk[pp - 1]
                    e = st.tile([128, N], f16, tag="e", name=f"e_{pp - 1}")
                    sume = st.tile([128, 1], f32, tag="sume", name=f"su_{pp - 1}")
                    nc.scalar.activation(
                        e[:], b["lg"][:], A.Exp, bias=b["nm"][:],
                        accum_out=sume[:],
                    )
                    b["sume"] = sume
                    dgall = dgp.tile(
                        [128, N * 128], f16, tag="dg", name=f"dg_{pp - 1}"
                    )
                    nc.gpsimd.local_scatter(
                        dgall[:], e[:], didxt[:],
                        channels=128, num_elems=N * 128, num_idxs=N,
                    )
                    acc_ps = ps.tile([128, D], f32, tag="acc", name=f"ps_{pp - 1}")
                    tq = b["t"]
                    for n in range(N):
                        dg_n = dgall[:, n * 128 : (n + 1) * 128]
                        nc.tensor.matmul(
                            acc_ps[:, 0:512], dg_n,
                            tq[:, n * D : n * D + 512],
                            start=(n == 0), stop=(n == N - 1),
                        )
                        nc.tensor.matmul(
                            acc_ps[:, 512:1024], dg_n,
                            tq[:, n * D + 512 : (n + 1) * D],
                            start=(n == 0), stop=(n == N - 1),
                        )
                    b["ps"] = acc_ps
                if pp < PB:
                    sos_eng = SOS_ENG[pp]
                    t = vp.tile([128, ND], f16, tag="v", name=f"v_{pp}")
                    if pp == 0:
                        # split the cold-start DMA so the first squares can
                        # begin earlier; small constant loads follow behind
                        for q in range(4):
                            nc.sync.dma_start(
                                t[:, q * (ND // 4) : (q + 1) * (ND // 4)],
                                v[pp, :, q * (ND // 4) : (q + 1) * (ND // 4)],
                            )
                        nc.sync.dma_start(wt[:], wsb[:])
                        nc.sync.dma_start(idt[:], ident[:])
                        nc.sync.dma_start(didxt[:], didx[:])
                    else:
                        nc.sync.dma_start(t[:], v[pp, :, :])
                    ss = st.tile([128, N], f32, tag="ss", name=f"ss_{pp}")
                    dot = st.tile([128, N], f32, tag="dot", name=f"dot_{pp}")
                    for n in range(N):
                        vn = t[:, n * D : (n + 1) * D]
                        if sos_eng[n] == "A":
                            sq = scrA.tile([128, D], f16, tag="sqA")
                            nc.scalar.activation(
                                sq[:], vn, A.Square,
                                accum_out=ss[:, n : n + 1],
                            )
                        else:
                            sq = scrV.tile([128, D], f16, tag="sqV")
                            nc.vector.scalar_tensor_tensor(
                                out=sq[:], in0=vn, scalar=1.0, in1=vn,
                                op0=OP.mult, op1=OP.mult,
                                accum_out=ss[:, n : n + 1],
                            )
                        td = scrV.tile([128, D], f16, tag="tdV")
                        nc.vector.scalar_tensor_tensor(
                            out=td[:], in0=vn, scalar=1.0, in1=wt[:],
                            op0=OP.mult, op1=OP.mult,
                            accum_out=dot[:, n : n + 1],
                        )
                    lnt = st.tile([128, N], f32, tag="lnt", name=f"ln_{pp}")
                    nc.scalar.activation(
                        lnt[:], ss[:], A.Ln, bias=epsb[:], scale=1.0 / D
                    )
                    y0 = st.tile([128, N], f32, tag="y0", name=f"y0_{pp}")
                    nc.scalar.activation(y0[:], lnt[:], A.Exp, scale=-0.5)
                    blk[pp] = {
                        "t": t, "dot": dot, "y0": y0,
                        "lg": st.tile([128, N], f32, tag="lg", name=f"lg_{pp}"),
                        "nm": st.tile([128, 1], f32, tag="nm", name=f"nm_{pp}"),
                    }
                if pp >= 1 and pp - 1 < PB:
                    b = blk[pp - 1]
                    rs = st.tile([128, 1], f32, tag="rs", name=f"rs_{pp - 1}")
                    nc.vector.reciprocal(rs[:], b["sume"][:])
                    b["rs"] = rs
                if pp < PB:
                    b = blk[pp]
                    nc.vector.tensor_mul(b["lg"][:], b["dot"][:], b["y0"][:])
                    nc.vector.tensor_reduce(
                        b["nm"][:], b["lg"][:], X, OP.max, negate=True
                    )
                if pp >= 2:
                    qp = pp - 2
                    b = blk.pop(qp)
                    acc = ac.tile([128, D], f16, tag="acc_sb")
                    nc.scalar.activation(
                        acc[:], b["ps"][:], A.Copy, scale=b["rs"][:]
                    )
                    nc.sync.dma_start(o[qp * 128 : (qp + 1) * 128, :], acc[:])

    nc.compile()
    return nc


def get_program():
    if "nc" not in _cache:
        _cache["nc"] = _build()
    return _cache["nc"]


def make_in_maps(V, proj, scale, block_idx):
    V = np.asarray(V, dtype=np.float32)
    proj = np.asarray(proj, dtype=np.float32)
    scale = np.asarray(scale, dtype=np.float32)
    idx = min(int(block_idx), proj.shape[0] - 1)
    ws = (proj[idx] * scale).astype(np.float16)
    wsb = np.ascontiguousarray(np.broadcast_to(ws, (128, D)))
    eye = np.eye(128, dtype=np.float16)
    didx = (
        np.arange(N, dtype=np.int16)[None, :] * 128
        + np.arange(128, dtype=np.int16)[:, None]
    ).astype(np.int16)
    # [N, BS, D] -> [NCORES, PB, 128, N, D] fp16
    Vp = (
        V.reshape(N, NCORES, PB, 128, D)
        .transpose(1, 2, 3, 0, 4)
        .astype(np.float16)
    )
    return [
        {
            "v": np.ascontiguousarray(Vp[k]).reshape(PB, 128, ND),
            "wsb": wsb,
            "ident": eye,
            "didx": didx,
        }
        for k in range(NCORES)
    ]


def kernel(V, proj, scale, block_idx):
    from concourse.bass_utils import run_bass_kernel_spmd

    nc = get_program()
    in_maps = make_in_maps(V, proj, scale, block_idx)
    res = run_bass_kernel_spmd(nc, in_maps, core_ids=list(range(NCORES)))
    _cache["last_exec_time_ns"] = res.exec_time_ns
    _cache["last_results"] = res
    out = np.concatenate(
        [res.results[k]["o"].astype(np.float32) for k in range(NCORES)], axis=0
    )
    return out.reshape(B, S, D)


# revision 17
# speedup vs baseline: 1.0623x; 1.0360x over previous
"""Trainium2 Bass kernel for nn_AttentionResidual (sparse_attention).

Computes, for V:(n=8,b=4,s=2048,d=1024), proj:(12,1024), scale:(1024,), block_idx:
    w       = proj[min(block_idx, 11)]
    rms     = sqrt(mean(V^2, axis=-1) + 1e-5)
    logits  = sum_d (w*scale)[d] * V[...,d] / rms
    weights = softmax(logits, axis=n)
    out     = sum_n weights[n] * V[n]                       # (b,s,d)

Sharding: data-parallel over the 8192 (b,s) positions across 8 NeuronCores
(1024 positions per core). proj/scale fold into one d-vector on the host.

v2 design (fp16): V is shipped fp16 in [block, pos, n, d] layout so each
128-position block is ONE 2 MiB DMA (16 KiB/partition contiguous). This
halves HBM traffic (the f32 roofline was ~107us; fp16 is ~53us) at ~7e-3
relative error, well inside the 2e-2 gate. Per block:
  - sum-of-squares and ws-dot reductions are split across ACT (Square+accum),
    DVE (tensor_scalar pow / STT + accum), and GPSIMD/Pool (STT+accum)
    per the SOS_ENG/DOT_ENG tables (DVE STT is 1x regardless of dtype;
    ACT has no 16-bit speedup; Pool is ~2x slower than DVE 1x - so the
    three-way split is what hides compute under the DMA floor).
  - softmax stats on [128,8] tiles (ACT Ln/Exp chain + DVE smalls)
  - weighted sum on the TensorEngine: diag(e_n) built by DVE tensor_scalar
    (4x fp16), 8 accumulating fp16 matmuls per PSUM half-bank pair; ACT
    copies PSUM->SBUF fp16 with the 1/sum(e) normalization folded in.
  - output DMA'd fp16 (host upcasts).
"""

import numpy as np

N, B, S, D = 8, 4, 2048, 1024
NCORES = 8
BS = B * S            # 8192 flattened (b,s) positions
PER = BS // NCORES    # 1024 positions per core
PB = PER // 128       # 8 position blocks per core
ND = N * D            # 8192 (n,d) elements per position
EPS = 1e-5

# Per-(block parity, n) engine assignment. A=ACT, V=DVE, P=Pool.
# Measured unit costs (fp16 [128,1024]): ACT Square+accum ~1266ns;
# DVE STT+accum ~1224ns (STT has no 2x modes); Pool TT-mult (~2.2us) +
# Pool tensor_reduce (~1us) - slow but the engine is otherwise idle.
SOS_ENG = ["AAAAAAAA"] * 3 + ["VAAAAAAA"] * 5  # per block
DOT_ENG = ["VVVVVVVV"] * 8

_cache = {}


def _build():
    import concourse.tile as tile
    from concourse import bacc, mybir

    OP = mybir.AluOpType
    A = mybir.ActivationFunctionType
    X = mybir.AxisListType.X
    f32 = mybir.dt.float32
    f16 = mybir.dt.float16

    from concourse.hw_specs import get_activation_tables

    nc = bacc.Bacc(
        "TRN2",
        target_bir_lowering=False,
        debug=False,
        enable_asserts=False,
        num_devices=NCORES,
    )
    v = nc.dram_tensor("v", [PB, 128, ND], f16, kind="ExternalInput").ap()
    wsb = nc.dram_tensor("wsb", [128, D], f16, kind="ExternalInput").ap()
    ident = nc.dram_tensor("ident", [128, 128], f16, kind="ExternalInput").ap()
    didx = nc.dram_tensor("didx", [128, N], mybir.dt.int16, kind="ExternalInput").ap()
    o = nc.dram_tensor("o", [PER, D], f16, kind="ExternalOutput").ap()

    # One ACT table set covers Square/Ln/Exp/Copy; pre-place its load so the
    # bacc pass doesn't ping-pong between smaller sets.
    act_set_id = list(get_activation_tables(nc.m.arch).keys()).index(
        "natural_log_exp_and_others"
    )

    with tile.TileContext(nc) as tc:
        with (
            tc.tile_pool(name="vp", bufs=4) as vp,
            tc.tile_pool(name="wp", bufs=1) as wp,
            tc.tile_pool(name="scrA", bufs=2) as scrA,
            tc.tile_pool(name="scrV", bufs=2) as scrV,
            tc.tile_pool(name="scrP", bufs=2) as scrP,
            tc.tile_pool(name="st", bufs=8) as st,
            tc.tile_pool(name="dg", bufs=3) as dgp,
            tc.tile_pool(name="ac", bufs=3) as ac,
            tc.tile_pool(name="ps", bufs=3, space="PSUM") as ps,
        ):
            nc.scalar.add_instruction(
                mybir.InstLoadActFuncSet(
                    name=nc.get_next_instruction_name(),
                    ins=[],
                    outs=[],
                    act_func_set_id=act_set_id,
                )
            )
            wt = wp.tile([128, D], f16, tag="w")
            idt = wp.tile([128, 128], f16, tag="id")
            didxt = wp.tile([128, N], mybir.dt.int16, tag="didx")
            epsb = wp.tile([128, 1], f32, tag="eps")
            nc.vector.memset(epsb[:], EPS)

            # Skewed software pipeline, one iteration per 128-position
            # block. In-order engine queues mean a dependency ping-pong
            # (ss -> Ln -> y0 -> lg -> nm -> e -> scatter -> matmul -> copy)
            # stalls every engine if issued densely per block; instead each
            # stage is issued one block behind the stage it depends on, so
            # every queued op's inputs are already complete when reached:
            #   iter pp: DVE[lg,nm(pp-1)] | reductions(pp) | DVE[rs(pp-1)]
            #            ACT[Ln,y0(pp)] ACT[e(pp-1)] Pool[scatter(pp-1)]
            #            PE[matmuls(pp-1)] ACT[copy(pp-2)]
            blk = {}
            for pp in range(PB + 2):
                if pp >= 1 and pp - 1 < PB:
                    b = blk[pp - 1]
                    e = st.tile([128, N], f16, tag="e", name=f"e_{pp - 1}")
                    sume = st.tile([128, 1], f32, tag="sume", name=f"su_{pp - 1}")
                    nc.scalar.activation(
                        e[:], b["lg"][:], A.Exp, bias=b["nm"][:],
                        accum_out=sume[:],
                    )
                    b["sume"] = sume
                    dgall = dgp.tile(
                        [128, N * 128], f16, tag="dg", name=f"dg_{pp - 1}"
                    )
                    nc.gpsimd.local_scatter(
                        dgall[:], e[:], didxt[:],
                        channels=128, num_elems=N * 128, num_idxs=N,
                    )
                    acc_ps = ps.tile([128, D], f32, tag="acc", name=f"ps_{pp - 1}")
                    tq = b["t"]
                    # bank0 fully first so its PSUM half (and the copy-out)
                    # can drain while bank1 still accumulates
                    for h in range(2):
                        for n in range(N):
                            nc.tensor.matmul(
                                acc_ps[:, h * 512 : (h + 1) * 512],
                                dgall[:, n * 128 : (n + 1) * 128],
                                tq[:, n * D + h * 512 : n * D + (h + 1) * 512],
                                start=(n == 0), stop=(n == N - 1),
                            )
                    b["ps"] = acc_ps
                if pp < PB:
                    sos_eng = SOS_ENG[pp]
                    t = vp.tile([128, ND], f16, tag="v", name=f"v_{pp}")
                    if pp == 0:
                        # split the cold-start DMA so the first squares can
                        # begin earlier; small constant loads follow behind
                        for q in range(4):
                            nc.sync.dma_start(
                                t[:, q * (ND // 4) : (q + 1) * (ND // 4)],
                                v[pp, :, q * (ND // 4) : (q + 1) * (ND // 4)],
                            )
                        nc.sync.dma_start(wt[:], wsb[:])
                        nc.sync.dma_start(idt[:], ident[:])
                        nc.sync.dma_start(didxt[:], didx[:])
                    else:
                        nc.sync.dma_start(t[:], v[pp, :, :])
                    ss = st.tile([128, N], f32, tag="ss", name=f"ss_{pp}")
                    dot = st.tile([128, N], f32, tag="dot", name=f"dot_{pp}")
                    for n in range(N):
                        vn = t[:, n * D : (n + 1) * D]
                        if sos_eng[n] == "A":
                            sq = scrA.tile([128, D], f16, tag="sqA")
                            nc.scalar.activation(
                                sq[:], vn, A.Square,
                                accum_out=ss[:, n : n + 1],
                            )
                        else:
                            sq = scrV.tile([128, D], f16, tag="sqV")
                            nc.vector.scalar_tensor_tensor(
                                out=sq[:], in0=vn, scalar=1.0, in1=vn,
                                op0=OP.mult, op1=OP.mult,
                                accum_out=ss[:, n : n + 1],
                            )
                        td = scrV.tile([128, D], f16, tag="tdV")
                        nc.vector.scalar_tensor_tensor(
                            out=td[:], in0=vn, scalar=1.0, in1=wt[:],
                            op0=OP.mult, op1=OP.mult,
                            accum_out=dot[:, n : n + 1],
                        )
                    lnt = st.tile([128, N], f32, tag="lnt", name=f"ln_{pp}")
                    nc.scalar.activation(
                        lnt[:], ss[:], A.Ln, bias=epsb[:], scale=1.0 / D
                    )
                    y0 = st.tile([128, N], f32, tag="y0", name=f"y0_{pp}")
                    nc.scalar.activation(y0[:], lnt[:], A.Exp, scale=-0.5)
                    blk[pp] = {
                        "t": t, "dot": dot, "y0": y0,
                        "lg": st.tile([128, N], f32, tag="lg", name=f"lg_{pp}"),
                        "nm": st.tile([128, 1], f32, tag="nm", name=f"nm_{pp}"),
                    }
                if pp >= 1 and pp - 1 < PB:
                    b = blk[pp - 1]
                    rs = st.tile([128, 1], f32, tag="rs", name=f"rs_{pp - 1}")
                    nc.vector.reciprocal(rs[:], b["sume"][:])
                    b["rs"] = rs
                if pp < PB:
                    b = blk[pp]
                    nc.vector.tensor_mul(b["lg"][:], b["dot"][:], b["y0"][:])
                    nc.vector.tensor_reduce(
                        b["nm"][:], b["lg"][:], X, OP.max, negate=True
                    )
                if pp >= 2:
                    qp = pp - 2
                    b = blk.pop(qp)
                    acc = ac.tile([128, D], f16, tag="acc_sb")
                    for h in range(2):
                        nc.scalar.activation(
                            acc[:, h * 512 : (h + 1) * 512],
                            b["ps"][:, h * 512 : (h + 1) * 512],
                            A.Copy, scale=b["rs"][:],
                        )
                    nc.sync.dma_start(o[qp * 128 : (qp + 1) * 128, :], acc[:])

    nc.compile()
    return nc


def get_program():
    if "nc" not in _cache:
        _cache["nc"] = _build()
    return _cache["nc"]


def make_in_maps(V, proj, scale, block_idx):
    V = np.asarray(V, dtype=np.float32)
    proj = np.asarray(proj, dtype=np.float32)
    scale = np.asarray(scale, dtype=np.float32)
    idx = min(int(block_idx), proj.shape[0] - 1)
    ws = (proj[idx] * scale).astype(np.float16)
    wsb = np.ascontiguousarray(np.broadcast_to(ws, (128, D)))
    eye = np.eye(128, dtype=np.float16)
    didx = (
        np.arange(N, dtype=np.int16)[None, :] * 128
        + np.arange(128, dtype=np.int16)[:, None]
    ).astype(np.int16)
    # [N, BS, D] -> [NCORES, PB, 128, N, D] fp16
    Vp = (
        V.reshape(N, NCORES, PB, 128, D)
        .transpose(1, 2, 3, 0, 4)
        .astype(np.float16)
    )
    return [
        {
            "v": np.ascontiguousarray(Vp[k]).reshape(PB, 128, ND),
            "wsb": wsb,
            "ident": eye,
            "didx": didx,
        }
        for k in range(NCORES)
    ]


def kernel(V, proj, scale, block_idx):
    from concourse.bass_utils import run_bass_kernel_spmd

    nc = get_program()
    in_maps = make_in_maps(V, proj, scale, block_idx)
    res = run_bass_kernel_spmd(nc, in_maps, core_ids=list(range(NCORES)))
    _cache["last_exec_time_ns"] = res.exec_time_ns
    _cache["last_results"] = res
    out = np.concatenate(
        [res.results[k]["o"].astype(np.float32) for k in range(NCORES)], axis=0
    )
    return out.reshape(B, S, D)
